# revision 18
# baseline (speedup 1.0000x reference)
"""Symmetric-decomposition variant of the EnhancedBalSCL TRN2 kernel.

raw = F F^T is symmetric, so each unordered 512x512 block pair is computed
ONCE: core c computes its diagonal block plus off-diagonal blocks against
cores c+1, c+2, c+3 (mod 8), and cores 0..3 also compute the {c, c+4} pair
block (cores 4..7 run a zero-padded dummy there to keep the SPMD program
uniform).  Each off-diagonal block yields BOTH:
  - row-sums   sum_k w_k exp(10 raw[i,k])  (DVE STT, as before) -> denom_i
  - col-sums   sum_i w_i exp(10 raw[i,k])  (PE ones-matmul with the w column
    as stationary, accumulated over the 4 row-tiles in PSUM)       -> denom_k
The host scatters the col-sums into the other cores' denominators.  Batch
columns per core drop from 4096 to 2560 (2048 real on cores 4..7), cutting
PE matmul count from 172 to 128 and total DMA from 6.6MiB to ~4.4MiB.

Everything else (fp8 DoubleRow raw blocks, bf16 exp via ACT -> SBUF, host-side
P/log/assembly, exact diag-term cancellation) matches kernel.py.

Device outputs per core:
  out  [128, 16] f32: row-sum partials, col 4m+u, u in {diag, pairA, pairB,
       centers} for row-tile m
  out2 [4, 512]  f32: col-sums of the 4 off-diag blocks (q -> block c+1+q),
       read from PSUM partitions 0/32/64/96
"""

import numpy as np
import ml_dtypes

_B, _D, _C, _M = 4096, 1024, 1000, 8
_BL = _B // _M            # 512 rows per core
_RT = _BL // 128          # 4 row tiles per core
_JT = _D // 256           # 4 super-K tiles (fp8 DoubleRow path)
_XC = 2048                # off-diagonal rhs columns per core (4 blocks)
_CP = 1024                # padded class dim
_W = _BL + _XC + _CP      # per-core w row: own | off-blocks | centers
_SCALE = 10.0             # 1/tau

_CACHE = {}


def _build_nc(reps=1):
    import concourse.bass as bass
    import concourse.mybir as mybir
    from concourse import bacc, tile
    from contextlib import ExitStack

    f32 = mybir.dt.float32
    bf16 = mybir.dt.bfloat16
    fp8 = mybir.dt.float8e4
    DR = mybir.MatmulPerfMode.DoubleRow
    AF = mybir.ActivationFunctionType
    OP = mybir.AluOpType

    nc = bacc.Bacc("TRN2", target_bir_lowering=False, debug=False,
                   num_devices=_M)
    fs_d = nc.declare_dram_parameter("fts", [_JT, 2, 128, _XC], fp8, isOutput=False)
    l8_d = nc.declare_dram_parameter("fl8", [_JT, 2, 128, _BL], fp8, isOutput=False)
    rc_d = nc.declare_dram_parameter("rc8", [_JT, 2, 128, _CP], fp8, isOutput=False)
    wv_d = nc.declare_dram_parameter("wv", [128, _W], bf16, isOutput=False)
    wc_d = nc.declare_dram_parameter("wcol", [128, _RT], bf16, isOutput=False)
    out_d = nc.declare_dram_parameter("out", [128, 16], f32, isOutput=True)
    o2_d = nc.declare_dram_parameter("out2", [4, 512], f32, isOutput=True)

    with tile.TileContext(nc) as tc, ExitStack() as ctx:
        consts = ctx.enter_context(tc.tile_pool(name="consts", bufs=1))
        psum = ctx.enter_context(tc.tile_pool(name="psum", bufs=1, space="PSUM"))
        xps = ctx.enter_context(tc.tile_pool(name="xps", bufs=4))
        scs = ctx.enter_context(tc.tile_pool(name="scs", bufs=3))

        fl8 = consts.tile([128, _JT * 2 * _BL], fp8, tag="fl8")
        fts = consts.tile([128, _JT * 2 * _XC], fp8, tag="fts")
        wv = consts.tile([128, _W], bf16, tag="wv")
        rc8t = consts.tile([128, _JT * 2 * _CP], fp8, tag="rct8")
        wcol = consts.tile([128, _RT], bf16, tag="wcol")
        outt = consts.tile([128, 16], f32, tag="outt")
        o2s = consts.tile([128, 512], f32, tag="o2s")

        def fl8_chunk(j, eng):
            eng.dma_start(
                fl8[:, j * 2 * _BL:(j + 1) * 2 * _BL].rearrange(
                    "p (i c) -> p i c", i=2),
                l8_d[j].rearrange("i p c -> p i c"))

        def fts_chunk(j, g, eng):
            # column-group g (0: cols 0:1024, 1: cols 1024:2048) of k-tile j
            eng.dma_start(
                fts[:, j * 2 * _XC:(j + 1) * 2 * _XC]
                .rearrange("p (i c) -> p i c", i=2)[:, :, g * 1024:(g + 1) * 1024],
                fs_d[j, :, :, g * 1024:(g + 1) * 1024].rearrange("i p c -> p i c"))

        def rc8_chunk(j, eng):
            eng.dma_start(
                rc8t[:, j * 2 * _CP:(j + 1) * 2 * _CP].rearrange(
                    "p (i c) -> p i c", i=2),
                rc_d[j].rearrange("i p c -> p i c"))

        # sync queue: fl8 j0/j1 (diag gate), then fts groups in use order
        fl8_chunk(0, nc.sync)
        fl8_chunk(1, nc.sync)
        for j in range(_JT):
            fts_chunk(j, 0, nc.sync)
        for j in range(_JT):
            fts_chunk(j, 1, nc.sync)
        rc8_chunk(2, nc.sync)
        rc8_chunk(3, nc.sync)

        # gpsimd queue: fl8 j2/j3, the w row (diag slice first), wcol, rc8
        fl8_chunk(2, nc.gpsimd)
        fl8_chunk(3, nc.gpsimd)
        nc.gpsimd.dma_start(wv[:, 0:_BL], wv_d[:, 0:_BL])
        nc.gpsimd.dma_start(wcol[:], wc_d[:])
        for s in range(3):
            lo = _BL + s * 1024
            nc.gpsimd.dma_start(wv[:, lo:lo + 1024], wv_d[:, lo:lo + 1024])
        rc8_chunk(0, nc.gpsimd)
        rc8_chunk(1, nc.gpsimd)

        lhs8 = [[fl8[:, j * 2 * _BL:(j + 1) * 2 * _BL]
                 .rearrange("p (i c) -> p i c", i=2)[:, :, m * 128:(m + 1) * 128]
                 for j in range(_JT)] for m in range(_RT)]

        def blk():
            return psum.tile([128, 1024], f32, tag="blk", bufs=3, name="ps")

        def mm_cols(ps, off, m, src_tile, src_w, lo, width):
            # raw block [128, width] from columns lo:lo+width of src_tile
            for j in range(_JT):
                rj = src_tile[:, j * 2 * src_w:(j + 1) * 2 * src_w].rearrange(
                    "p (i c) -> p i c", i=2)
                for h in range(width // 512):
                    o = off + h * 512
                    s = lo + h * 512
                    nc.tensor.matmul(ps[:, o:o + 512], lhs8[m][j],
                                     rj[:, :, s:s + 512],
                                     start=(j == 0), stop=(j == _JT - 1),
                                     perf_mode=DR)

        def wsum(ps, width, wlo, acc_col):
            xp = xps.tile([128, 1024], bf16, tag="xp", bufs=4, name="xp")
            nc.scalar.activation(xp[:, :width], ps[:, :width], AF.Exp,
                                 scale=_SCALE)
            sc = scs.tile([128, 1024], bf16, tag="sc", bufs=3, name="sc")
            nc.vector.scalar_tensor_tensor(
                out=sc[:, :width], in0=xp[:, :width], scalar=1.0,
                in1=wv[:, wlo:wlo + width],
                op0=OP.mult, op1=OP.mult,
                accum_out=outt[:, acc_col:acc_col + 1])
            return xp

        def body(_i=None):
            cs = psum.tile([128, 512], f32, tag="cs", bufs=1, name="cs")
            cs2 = psum.tile([128, 512], f32, tag="cs2", bufs=1, name="cs2")

            deferred = []

            def drain(keep):
                # emit deferred col-sum matmuls lagged behind their block so
                # PE never waits on the ACT->SBUF exp ack latency
                while len(deferred) > keep:
                    deferred.pop(0)()

            def colsum_of(pair, m, xp):
                def emit():
                    for q in (2 * pair, 2 * pair + 1):
                        tgt = cs[32 * q:32 * q + 1, 0:512] if q < 3 \
                            else cs2[0:1, 0:512]
                        nc.tensor.matmul(
                            tgt, wcol[:, m:m + 1],
                            xp[:, (q % 2) * 512:(q % 2) * 512 + 512],
                            start=(m == 0), stop=(m == _RT - 1))
                return emit

            # diag unit m0..m2: needs only fl8, starts the pipeline early;
            # m3 runs last so the serial tail ends on a narrow 512 block
            for m in range(_RT - 1):
                ps = blk()
                mm_cols(ps, 0, m, fl8, _BL, 0, 512)
                wsum(ps, 512, 0, 4 * m)
            # off-diagonal pair units; col-sums accumulate in cs, lagged 2
            for pair in (0, 1):
                for m in range(_RT):
                    ps = blk()
                    mm_cols(ps, 0, m, fts, _XC, pair * 1024, 1024)
                    drain(2)
                    xp = wsum(ps, 1024, _BL + pair * 1024, 4 * m + 1 + pair)
                    deferred.append(colsum_of(pair, m, xp))
            # centers (remaining col-sums drain between the center blocks)
            for m in range(_RT):
                ps = blk()
                mm_cols(ps, 0, m, rc8t, _CP, 0, 1024)
                drain(1 - m if m < 2 else 0)
                wsum(ps, 1024, _BL + _XC, 4 * m + 3)
            drain(0)
            # stage the finished col-sums to SBUF (PSUM cannot be DMA'd)
            for q in range(3):
                nc.scalar.activation(o2s[32 * q:32 * q + 1, :],
                                     cs[32 * q:32 * q + 1, :], AF.Copy)
            nc.scalar.activation(o2s[96:97, :], cs2[0:1, :], AF.Copy)
            # diag m3: the short final block
            ps = blk()
            mm_cols(ps, 0, 3, fl8, _BL, 0, 512)
            wsum(ps, 512, 0, 12)

        if reps == 1:
            body()
        else:
            with tc.For_i(0, reps, 1,
                          hint_engines=(mybir.EngineType.PE,)) as i:
                body(i)

        # out2 rides the idle gpsimd queue as soon as the copies land
        # (mid-kernel); only outt's DMA trails the final block
        nc.gpsimd.dma_start(
            o2_d[0:3],
            o2s[0:96].rearrange("(a b) c -> a b c", b=32)[:, 0:1, :])
        nc.gpsimd.dma_start(o2_d[3:4], o2s[96:97, :])
        nc.sync.dma_start(out_d[:], outt[:])

    nc.compile()
    return nc


def _get_nc():
    if "nc" not in _CACHE:
        _CACHE["nc"] = _build_nc()
    return _CACHE["nc"]


def _prep_inputs(centers, features, targets):
    bf16 = ml_dtypes.bfloat16
    fp8 = ml_dtypes.float8_e4m3
    F = np.ascontiguousarray(features, dtype=np.float32)      # [B, D]
    Cen = np.ascontiguousarray(centers, dtype=np.float32)     # [C, D]
    t = np.asarray(targets).astype(np.int64).ravel()          # [B]

    counts = np.bincount(t, minlength=_C).astype(np.float32)
    w = (1.0 / (counts[t] + 1.0)).astype(np.float32)
    v = (1.0 / (counts + 1.0)).astype(np.float32)
    H = np.zeros((_C, _D), dtype=np.float32)
    np.add.at(H, t, F)

    F8 = F.astype(fp8)
    F8f = F8.astype(np.float32)
    FT8 = np.ascontiguousarray(F8.T)                          # [D, B] fp8
    CT8 = np.zeros((_D, _CP), dtype=fp8)
    CT8[:, :_C] = Cen.astype(fp8).T
    rc8 = np.ascontiguousarray(CT8.reshape(_JT, 2, 128, _CP))

    wb = w.astype(bf16)
    vb16 = v.astype(bf16)

    # host-side finals: positive term P and the diag-term cancellation
    U8f = (H + Cen).astype(fp8).astype(np.float32)
    P = np.einsum("bd,bd->b", F8f, U8f[t, :], dtype=np.float32)
    diag8 = np.einsum("bd,bd->b", F8f, F8f, dtype=np.float32)
    dev_diag = (np.exp(np.float32(_SCALE) * diag8).astype(bf16).astype(np.float32)
                * wb.astype(np.float32)).astype(bf16).astype(np.float32)
    q = (F * F).sum(axis=1)
    corr = w * np.exp(np.float32(_SCALE) * q) - dev_diag
    pos = (P - diag8) * (np.float32(_SCALE) / counts[t])

    def col(x_loc):
        return np.ascontiguousarray(x_loc.reshape(_RT, 128).T)

    in_maps = []
    host = []
    for c in range(_M):
        R = c * _BL
        fl8 = np.ascontiguousarray(FT8[:, R:R + _BL]).reshape(_JT, 2, 128, _BL)
        # off-diagonal rhs: blocks c+1, c+2, c+3 and (c<4) c+4, else zeros
        blocks = [(c + 1) % _M, (c + 2) % _M, (c + 3) % _M]
        if c < 4:
            blocks.append(c + 4)
        cols = np.concatenate([np.arange(b * _BL, (b + 1) * _BL) for b in blocks])
        fts = np.zeros((_D, _XC), dtype=fp8)
        fts[:, :len(cols)] = FT8[:, cols]
        fts = np.ascontiguousarray(fts.reshape(_JT, 2, 128, _XC))
        wv_row = np.zeros(_W, dtype=bf16)
        wv_row[0:_BL] = wb[R:R + _BL]
        wv_row[_BL:_BL + len(cols)] = wb[cols]
        wv_row[_BL + _XC:_BL + _XC + _C] = vb16
        wv_full = np.ascontiguousarray(np.broadcast_to(wv_row, (128, _W)))
        in_maps.append({
            "fts": fts, "fl8": fl8, "rc8": rc8, "wv": wv_full,
            "wcol": col(wb[R:R + _BL].astype(np.float32)).astype(bf16),
        })
        host.append({
            "corr": col(corr[R:R + _BL]),
            "pos": col(pos[R:R + _BL]),
            "blocks": blocks,
        })
    _CACHE["host"] = host
    return in_maps


def _assemble(results):
    host = _CACHE["host"]
    # scatter the col-sum contributions into global per-sample partials
    den_col = np.zeros(_B, dtype=np.float32)
    for c in range(_M):
        o2 = np.asarray(results[c]["out2"], dtype=np.float32)
        for qi, b in enumerate(host[c]["blocks"]):
            den_col[b * _BL:(b + 1) * _BL] += o2[qi]
    total = 0.0
    for c in range(_M):
        o = np.asarray(results[c]["out"], dtype=np.float32)
        h = host[c]
        den = o[:, 0::4] + o[:, 1::4] + o[:, 2::4] + o[:, 3::4]  # [128, RT]
        R = c * _BL
        den = den + den_col[R:R + _BL].reshape(_RT, 128).T + h["corr"]
        per = np.log(den) - h["pos"]
        total += float(per.sum())
    return np.float32(total / _B)


def _run(inputs, trace=False, **trace_kwargs):
    from concourse.bass_utils import run_bass_kernel_spmd
    nc = _get_nc()
    in_maps = _prep_inputs(**inputs)
    res = run_bass_kernel_spmd(nc, in_maps, core_ids=list(range(_M)),
                               trace=trace, **trace_kwargs)
    return _assemble(res.results), res


def kernel(centers, features, targets):
    out, _ = _run({"centers": centers, "features": features, "targets": targets})
    return out
